# revision 12
# baseline (speedup 1.0000x reference)
"""Trainium2 Bass kernel for nn_AttentionBlock (sparse causal attention).

Math (per batch b, head h), with A = r_prime[b] (T x N):
    Omega_h = tril(A @ Q_h @ A^T)            (T x T)
    out[b]  = sum_h Omega_h @ (A @ E_h^T)    (T x N)

Strategy: data-parallel over batch (8 batches -> 8 NeuronCores).  Per core,
a chunked linear-attention decomposition with chunk C=128 avoids the T x T
matrix: for row-chunk I,
    out_I = sum_h [ tril(A_I Q_h A_I^T) @ Er_{h,I}  +  A_I Q_h S_{h,I} ]
where Er_h = A E_h^T and S_{h,I} = sum_{J<I} A_J^T Er_{h,J} is an N x N
running state, accumulated directly in a PSUM bank by the PE.

All matmul inputs are fp16 (full PE rate), fp32 PSUM accumulation; the
kernel computes out^T (N x T) and the host transposes back.

Hardware quirks honored: matmul operands must be read from SBUF partition
base 0 (upper-half operand streaming is broken on this runtime), and PE
writes to PSUM partition offsets != 0 corrupt data.  Head-stacked operands
(which live at partitions 64..127) are therefore relocated with SBUF->SBUF
DMAs before being consumed, and wide-N matmuls (N=256 spanning chunk pairs
or head pairs via strided access patterns) keep the PE efficient.
"""

import numpy as np

import concourse.bacc as bacc
import concourse.bass as bass
import concourse.mybir as mybir
import concourse.tile as tile
from concourse.bass_utils import run_bass_kernel_spmd

D, T, N, H = 8, 2048, 64, 8
C = 128  # row-chunk size
F16 = mybir.dt.float16
F32 = mybir.dt.float32

# W psum banks hold heads in this column order (see W matmul pairing)
_WM_SLOT = {0: 0, 2: 1, 1: 2, 3: 3}


def build_nc(t_len: int = T, reps: int = 1, loop_reps: int = 1) -> bacc.Bacc:
    """Build the per-core Bass module.  reps>1 repeats the computation
    python-unrolled; loop_reps>1 wraps it in a hardware For_i loop instead
    (both produce identical output; used for wall-clock delta timing)."""
    nch = t_len // C
    assert nch % 2 == 0
    nc = bacc.Bacc("TRN2", target_bir_lowering=False, debug=False)

    rp = nc.dram_tensor("rp", [C, nch * N], F16, kind="ExternalInput")
    rpt = nc.dram_tensor("rpt", [N, t_len], F16, kind="ExternalInput")
    q_all = nc.dram_tensor("q_all", [N, H * N], F16, kind="ExternalInput")
    et_all = nc.dram_tensor("et_all", [N, H * N], F16, kind="ExternalInput")
    maskd = nc.dram_tensor("maskd", [C, 4 * C], F32, kind="ExternalInput")
    out_t = nc.dram_tensor("out_t", [N, t_len], F32, kind="ExternalOutput")

    with tile.TileContext(nc) as tc:
        with (
            tc.tile_pool(name="const", bufs=1) as cpool,
            tc.tile_pool(name="csb", bufs=2) as c_pool,
            tc.tile_pool(name="chib", bufs=2) as chi_pool,
            tc.tile_pool(name="ersb", bufs=2) as er_pool,
            tc.tile_pool(name="wm", bufs=3) as wm_pool,
            tc.tile_pool(name="s16p", bufs=2) as s16_pool,
            tc.tile_pool(name="ps_c", bufs=2, space="PSUM") as ps_c,
            tc.tile_pool(name="ps_w", bufs=2, space="PSUM") as ps_w,
            tc.tile_pool(name="ps_er", bufs=1, space="PSUM") as ps_er,
            tc.tile_pool(name="ps_s", bufs=1, space="PSUM") as ps_s,
            tc.tile_pool(name="ps_o", bufs=2, space="PSUM") as ps_o,
        ):
            # --- constants, loaded once ---
            rp_sb = cpool.tile([C, nch * N], F16)
            nc.sync.dma_start(rp_sb[:], rp[:])
            rpt_sb = cpool.tile([N, t_len], F16)
            nc.sync.dma_start(rpt_sb[:], rpt[:])
            q_sb = cpool.tile([N, H * N], F16)
            nc.sync.dma_start(q_sb[:], q_all[:])
            et_sb = cpool.tile([N, H * N], F16)
            nc.sync.dma_start(et_sb[:], et_all[:])
            mask_sb = cpool.tile([C, 4 * C], F32)
            nc.sync.dma_start(mask_sb[:], maskd[:])
            out_sb = cpool.tile([N, t_len], F32)
            # absorb the mask DMA wait on the DVE clock early
            scr = cpool.tile([1, 4], F32)
            nc.vector.tensor_copy(scr[:], mask_sb[:1, :4])

            def one_rep():
                p_s = None
                for ii in range(nch // 2):
                    psl = slice(ii * 2 * C, (ii + 1) * 2 * C)

                    # C_h = Q_h^T A^T for BOTH chunks of the pair (N=256),
                    # head-pairs stacked on output partitions (M=128):
                    # c2_sb[64e+k, 256p+128m+t] = C_{2p+e}[k,t], chunk m.
                    c2_sb = c_pool.tile([2 * N, 2 * 4 * C], F16, tag="c2")
                    for q in range(2):
                        p_c = ps_c.tile([2 * N, 4 * C], F32, tag="c")
                        for pp in range(2):
                            p = 2 * q + pp
                            nc.tensor.matmul(
                                p_c[:, pp * 2 * C : (pp + 1) * 2 * C],
                                lhsT=q_sb[:, p * 2 * N : (p + 1) * 2 * N],
                                rhs=rpt_sb[:, psl],
                                start=(pp == 0),
                                stop=(pp == 1),
                            )
                        nc.scalar.copy(
                            c2_sb[:, q * 4 * C : (q + 1) * 4 * C], p_c[:]
                        )
                    # relocate the e=1 head half to partition base 0
                    c_hi = chi_pool.tile([N, 2 * 4 * C], F16, tag="chi")
                    nc.sync.dma_start(c_hi[:], c2_sb[N : 2 * N, :])
                    c_lo = c2_sb[0 : N, :]

                    for m in range(2):
                        i = 2 * ii + m
                        tsl = slice(i * C, (i + 1) * C)

                        # Er_{h,I} = A_I E_h^T, all heads: p_er[u, 64h+i2]
                        p_er = ps_er.tile([C, H * N], F32, tag="er")
                        nc.tensor.matmul(
                            p_er[:], lhsT=rpt_sb[:, tsl], rhs=et_sb[:],
                            start=True, stop=True,
                        )
                        er_sb = er_pool.tile([C, H * N], F16, tag="er_sb")
                        nc.scalar.copy(er_sb[:], p_er[:])

                        # state snapshot S_I (before this chunk's P update),
                        # then DMA-stack head pairs for K=128 inter matmuls:
                        # s16_2[64e+k, 64p+i2] = S_{2p+e}[k,i2]
                        if i > 0:
                            s16f = s16_pool.tile([N, H * N], F16, tag="s16f")
                            nc.scalar.copy(s16f[:], p_s[:])
                            s16_2 = s16_pool.tile([2 * N, 4 * N], F16, tag="s16s")
                            s16f_v = s16f[:].rearrange(
                                "k (p e i2) -> k p e i2", p=4, e=2
                            )
                            for e in range(2):
                                nc.sync.dma_start(
                                    s16_2[e * N : (e + 1) * N, :].rearrange(
                                        "k (p i2) -> k p i2", p=4
                                    ),
                                    s16f_v[:, :, e, :],
                                )

                        # W_h = A_I C_h = Omega_h^T (u,t); each matmul covers
                        # two same-parity heads via a strided rhs (N=256).
                        # Bank q column order: heads [4q, 4q+2, 4q+1, 4q+3].
                        wm_tiles = []
                        for q in range(2):
                            p_w = ps_w.tile([C, 4 * C], F32, tag="w")
                            for e in range(2):
                                src = c_lo if e == 0 else c_hi[:]
                                c_v = src.rearrange(
                                    "k (p mm t) -> k p mm t", p=4, mm=2
                                )
                                nc.tensor.matmul(
                                    p_w[:, e * 2 * C : (e + 1) * 2 * C],
                                    lhsT=rpt_sb[:, tsl],
                                    rhs=c_v[:, 2 * q : 2 * q + 2, m, :],
                                    start=(e == 0),
                                    stop=(e == 1),
                                )
                            wm = wm_pool.tile([C, 4 * C], F16, tag="wm")
                            nc.vector.tensor_mul(wm[:], p_w[:], mask_sb[:])
                            wm_tiles.append(wm)

                        # out^T accumulation for this chunk
                        p_o = ps_o.tile([N, C], F32, tag="o")
                        n_groups = 8 if i == 0 else 12
                        g = 0
                        for h in range(H):
                            q, r = divmod(h, 4)
                            j = _WM_SLOT[h % 4]
                            nc.tensor.matmul(
                                p_o[:],
                                lhsT=er_sb[:, h * N : (h + 1) * N],
                                rhs=wm_tiles[q][:, j * C : (j + 1) * C],
                                start=(g == 0),
                                stop=(g == n_groups - 1),
                            )
                            g += 1
                        if i > 0:
                            # inter: (A_I Q_h S_h)^T = S_h^T C_h, two heads
                            # per matmul via the stacked s16_2/c2_sb layout
                            for p in range(4):
                                nc.tensor.matmul(
                                    p_o[:],
                                    lhsT=s16_2[:, p * N : (p + 1) * N],
                                    rhs=c2_sb[:, p * 2 * C + m * C :
                                              p * 2 * C + (m + 1) * C],
                                    start=False,
                                    stop=(g == n_groups - 1),
                                )
                                g += 1
                        nc.scalar.copy(out_sb[:, tsl], p_o[:])

                        # P_h = A_I^T Er_{h,I} accumulated into PSUM state
                        if i < nch - 1:
                            if i == 0:
                                p_s = ps_s.tile([N, H * N], F32, tag="s")
                            nc.tensor.matmul(
                                p_s[:],
                                lhsT=rp_sb[:, i * N : (i + 1) * N],
                                rhs=er_sb[:],
                                start=(i == 0),
                                stop=(i == nch - 2),
                                # state bank is read (s16 snapshot) between
                                # accumulating matmuls; still start-once/
                                # accumulate semantics
                                skip_group_check=True,
                            )

                nc.sync.dma_start(out_t[:], out_sb[:])

            if loop_reps > 1:
                with tc.For_i(
                    0, loop_reps, 1,
                    hint_engines=(
                        mybir.EngineType.PE,
                        mybir.EngineType.Activation,
                        mybir.EngineType.DVE,
                        mybir.EngineType.SP,
                    ),
                ):
                    one_rep()
            else:
                for _rep in range(reps):
                    one_rep()

    nc.compile()
    return nc


def _host_prep(r_prime: np.ndarray, Q: np.ndarray, E: np.ndarray, t_len: int = T):
    """Shard + lay out host-side inputs for each of the 8 cores."""
    nch = t_len // C
    # q_all[j, 64h+k] = Q[h, j, k]
    q_all = np.ascontiguousarray(Q.transpose(1, 0, 2).reshape(N, H * N)).astype(
        np.float16
    )
    # et_all[j, 64h+i2] = E[h, i2, j]
    et_all = np.ascontiguousarray(E.transpose(2, 0, 1).reshape(N, H * N)).astype(
        np.float16
    )
    mask = np.tile(np.triu(np.ones((C, C), np.float32)), (1, 4))
    in_maps = []
    for b in range(D):
        a = r_prime[b]  # (t_len, N)
        rp16 = (
            a.reshape(nch, C, N).transpose(1, 0, 2).reshape(C, nch * N)
        ).astype(np.float16)
        rpt16 = np.ascontiguousarray(a.T).astype(np.float16)
        in_maps.append(
            {
                "rp": rp16,
                "rpt": rpt16,
                "q_all": q_all,
                "et_all": et_all,
                "maskd": mask,
            }
        )
    return in_maps


_NC_CACHE: dict = {}


def kernel(r_prime: np.ndarray, Q: np.ndarray, E: np.ndarray) -> np.ndarray:
    r_prime = np.asarray(r_prime, np.float32)
    Q = np.asarray(Q, np.float32)
    E = np.asarray(E, np.float32)
    t_len = r_prime.shape[1]
    if ("nc", t_len) not in _NC_CACHE:
        _NC_CACHE[("nc", t_len)] = build_nc(t_len)
    nc = _NC_CACHE[("nc", t_len)]
    in_maps = _host_prep(r_prime, Q, E, t_len)
    res = run_bass_kernel_spmd(nc, in_maps, list(range(D)))
    out = np.stack(
        [np.ascontiguousarray(res.results[b]["out_t"].T) for b in range(D)]
    )
    return out.astype(np.float32)
